# revision 14
# baseline (speedup 1.0000x reference)
"""Trainium2 Bass kernel for nn_DecoderBlock (B=8, S=1024, D=256, H=4 heads
of full width 256, FF=1024).

Strategy: pure data parallelism — B=8 batch elements across 8 NeuronCores,
zero collectives. Per core, one full decoder block in "transposed" activation
layout (features on SBUF partitions, tokens on the free dim), with the heavy
matmuls in fp8e4 DoubleRow mode (2 fp8 MACs per PE cell per cycle):

  tT   = (A8)^T @ x8        per head, A = wq^T wk precomputed on host, so
                            scores = x A x^T needs no separate q/k projections
  v    = (x8)^T @ wv8       [tokens, E], stored as fp8 key-block pairs
  sp   = (x8_keys)^T @ tT   causal lower-triangle blocks only
  ek   = exp(sp * 2^-15)    fp8, causal triangle masked on GpSimd
  Z    = halves^T @ ek      softmax denominators (DoubleRow "ones" = 0.5 so
                            the reciprocal also fixes the v/ont scale)
  oT   = v^T @ ek ; ont = oT * (1/Z)  -> fp8 * 64
  mh   = wo8^T @ ont ; r1 = mh*2^-12 + (x^T + wo_b) ; LN1
  ffn  : h = relu(ff18^T @ x1b8 * 2^-10 + 16*ff1_b) fp8 *16
         r2 = ff28^T @ h * 2^-10 + ff2_b + x1 ; LN2 in 4 chunks of 256 so the
         final LayerNorm tail after the last matmul is short.

All fp8 operands carry power-of-2 scales (x*16, A*4096, w*64, ...) folded
into activation scales / host-prescaled biases; accumulation is fp32 in PSUM
and LayerNorm math is fp32 (stats matmuls bf16). The attention_mask input is
all ones per the problem spec (causal mask only); if a mask with zeros ever
shows up, we fall back to a numpy reference.
"""

import numpy as np
import ml_dtypes

import concourse.bass as bass
import concourse.mybir as mybir
import concourse.tile as tile
from concourse import bacc
from concourse.bass_utils import run_bass_kernel_spmd

F32 = mybir.dt.float32
BF16 = mybir.dt.bfloat16
F8 = mybir.dt.float8e4
AF = mybir.ActivationFunctionType
ALU = mybir.AluOpType
DR = mybir.MatmulPerfMode.DoubleRow

N_CORES = 8
B, S, D, H, E, HE, FF = 8, 1024, 256, 4, 256, 1024, 1024
SC = 512          # token (free-dim) chunk for attention / wo / ff1
NJ = S // SC      # 2 chunks
SC2 = 256         # token chunk for ff2 + LN2 (short tail)
NJ2 = S // SC2    # 4 chunks
ND = D // 128     # 2 partition chunks of features
NF = FF // 128    # 8 partition chunks of ff features
LN_EPS = 1e-5
EXP_SCALE = 2.0 ** -14  # scores psum = (16x)·(64t) = 1024·s ; softmax /16

_CACHE = {}


def _build():
    nc = bacc.Bacc("TRN2", target_bir_lowering=False, debug=False,
                   num_devices=N_CORES)

    # ---- DRAM parameters (per-core shard + replicated weights) ----
    x8_d = nc.dram_tensor("x8", [128, 2, S], F8, kind="ExternalInput")
    A8_d = nc.dram_tensor("A8", [128, 2, H * D], F8, kind="ExternalInput")
    wv8_d = nc.dram_tensor("wv8", [128, 2, HE], F8, kind="ExternalInput")
    bias_d = nc.dram_tensor("biasp", [128, 18], F32, kind="ExternalInput")
    wo8_d = nc.dram_tensor("wo8", [128, NF, D], F8, kind="ExternalInput")
    ff18_d = nc.dram_tensor("ff18", [128, 2, FF], F8, kind="ExternalInput")
    ff28_d = nc.dram_tensor("ff28", [128, NF, D], F8, kind="ExternalInput")
    xres_d = nc.dram_tensor("xres", [ND, 128, S], F32, kind="ExternalInput")
    out_d = nc.dram_tensor("out", [ND, 128, S], F32, kind="ExternalOutput")

    with tile.TileContext(nc) as tc:
        with tc.tile_pool(name="consts", bufs=1) as consts, \
             tc.tile_pool(name="acts", bufs=1) as acts, \
             tc.tile_pool(name="work", bufs=2) as work, \
             tc.tile_pool(name="lnp", bufs=1) as lnp, \
             tc.tile_pool(name="psA", bufs=3, space="PSUM") as psA, \
             tc.tile_pool(name="psO", bufs=3, space="PSUM") as psO, \
             tc.tile_pool(name="psZ", bufs=2, space="PSUM") as psZ:

            def loadc(dram, shape, dt):
                t = consts.tile(shape, dt, tag=dram.name, name=dram.name)
                nc.sync.dma_start(out=t[:], in_=dram[:])
                return t

            # DMA in first-use order so the PE can start ASAP
            x8 = loadc(x8_d, [128, 2, S], F8)
            A8 = loadc(A8_d, [128, 2, H * D], F8)
            wv8 = loadc(wv8_d, [128, 2, HE], F8)
            biasp = loadc(bias_d, [128, 18], F32)
            wo8 = loadc(wo8_d, [128, NF, D], F8)
            ff18 = loadc(ff18_d, [128, 2, FF], F8)
            ff28 = loadc(ff28_d, [128, NF, D], F8)
            xres = []
            for i in range(ND):
                t = consts.tile([128, S], F32, tag=f"xres{i}", name=f"xres{i}")
                nc.sync.dma_start(out=t[:], in_=xres_d[i])
                xres.append(t)

            # bias pack columns
            ff1b16 = [biasp[:, f0:f0 + 1] for f0 in range(NF)]
            ff2_b = [biasp[:, 8 + i:9 + i] for i in range(ND)]
            ln1_g = [biasp[:, 10 + i:11 + i] for i in range(ND)]
            ln1_b = [biasp[:, 12 + i:13 + i] for i in range(ND)]
            ln2_g = [biasp[:, 14 + i:15 + i] for i in range(ND)]
            ln2_b = [biasp[:, 16 + i:17 + i] for i in range(ND)]

            # "halves" matrix for the Z matmul: DoubleRow over key-block
            # pairs with value 0.5, so Z' = Z/2 and 1/Z' folds the v(32x) and
            # ont(64x) scales into the existing reciprocal. [128,128] result
            # is replicated on all partitions.
            ones8 = consts.tile([128, 2, 128], F8, tag="ones8", name="ones8")
            nc.vector.memset(ones8[:], 0.5)
            onesb = consts.tile([128, 128], BF16, tag="onesb", name="onesb")
            nc.vector.memset(onesb[:], 1.0)
            invd_bf = consts.tile([128, 128], BF16, tag="invd", name="invd")
            nc.vector.memset(invd_bf[:], 1.0 / D)  # 2^-8, exact in bf16
            eps_t = consts.tile([128, 1], F32, tag="eps", name="eps")
            nc.vector.memset(eps_t[:], LN_EPS)
            sA = consts.tile([128, 1], F32, tag="sA", name="sA")
            nc.vector.memset(sA[:], 2.0 ** -12)   # wo psum scale
            sB = consts.tile([128, 1], F32, tag="sB", name="sB")
            nc.vector.memset(sB[:], 2.0 ** -10)   # ff2 psum scale
            # multiplicative causal mask for the diagonal 128x128 block of a
            # transposed [t, s] exp tile: 1 where t <= s else 0
            cmaskf = consts.tile([128, 128], F32, tag="cmaskf", name="cmaskf")
            nc.gpsimd.memset(cmaskf[:], 1.0)
            nc.gpsimd.affine_select(
                out=cmaskf[:], in_=cmaskf[:],
                compare_op=ALU.is_ge, fill=0.0,
                base=0, pattern=[[1, 128]], channel_multiplier=-1,
            )
            cmask8 = consts.tile([128, 128], F8, tag="cmask8", name="cmask8")
            nc.gpsimd.tensor_copy(out=cmask8[:], in_=cmaskf[:])
            # [zeros | triangle] combo for the upper block of a diagonal
            # pair: zeroes the fully-masked strip and masks the diagonal in
            # one gpsimd multiply (op-type switches cost library reloads)
            cmask28 = consts.tile([128, 256], F8, tag="cmask28", name="cmask28")
            nc.gpsimd.memset(cmask28[:, 0:128], 0.0)
            nc.gpsimd.tensor_copy(out=cmask28[:, 128:256], in_=cmaskf[:])

            # PE warmup: dummy matmuls (no DMA dependency) keep the HAM
            # clock-gate at 8/8 across known PE-idle bubbles.
            def warm_pe(n):
                for _ in range(n):
                    wp = psA.tile([128, 128], F32, tag="mm", name="warm")
                    nc.tensor.matmul(wp[:], onesb[:], onesb[:],
                                     start=True, stop=True)

            warm_pe(26)

            # ---- attention: per head -> ont8 [128, 2H, S] fp8 (64*o) ----
            ont8 = acts.tile([128, 2 * H, S], F8, tag="ont8", name="ont8")

            def make_head(h):
                """Allocate head h's tT/v tiles and return thunks that emit
                one projection matmul-group each, in first-use order."""
                tT8 = work.tile([128, 2, S], F8, tag="tT", name=f"tT{h}")
                vp8 = [work.tile([128, 2, E], F8, tag=f"vp{p}",
                                 name=f"vp{h}_{p}") for p in range(4)]

                def t_group(e0, j):
                    cols = slice(j * SC, (j + 1) * SC)
                    p = psA.tile([128, SC], F32, tag="mm", name="mm")
                    nc.tensor.matmul(
                        p[:], A8[:, :, h * D + e0 * 128: h * D + (e0 + 1) * 128],
                        x8[:, :, cols], start=True, stop=True, perf_mode=DR)
                    nc.scalar.activation(out=tT8[:, e0, cols], in_=p[:],
                                         func=AF.Copy, scale=2.0 ** -9)

                def v_group(pr):
                    p = psA.tile([128, 2, E], F32, tag="mm", name="mm")
                    for i in range(2):
                        t0 = 2 * pr + i
                        nc.tensor.matmul(
                            p[:, i, :], x8[:, :, t0 * 128:(t0 + 1) * 128],
                            wv8[:, :, h * E:(h + 1) * E],
                            start=True, stop=True, perf_mode=DR)
                    nc.vector.tensor_scalar_mul(
                        out=vp8[pr][:, :, :], in0=p[:, :, :],
                        scalar1=1.0 / 16.0)

                thunks = [lambda e0=0: t_group(0, 0), lambda: v_group(0),
                          lambda e0=1: t_group(1, 0), lambda: v_group(1),
                          lambda: t_group(0, 1), lambda: v_group(2),
                          lambda: t_group(1, 1), lambda: v_group(3)]
                return tT8, vp8, thunks

            cur = make_head(0)
            for t in cur[2]:
                t()

            for h in range(H):
                tT8, vp8, _ = cur
                nxt = make_head(h + 1) if h + 1 < H else None
                pending = list(nxt[2]) if nxt else []
                n_iters = 6  # total pair iterations this head
                it = 0
                done = 0
                zp = [psZ.tile([128, SC], F32, tag="z", name="z")
                      for j in range(NJ)]
                op = [[psO.tile([128, SC], F32, tag="o", name="o")
                       for _ in range(2)] for j in range(NJ)]
                for j in range(NJ):
                    npair = (4 * j + 4) // 2
                    pend = []

                    def emit_zav(item):
                        jj, pr, ek, off, w = item
                        last = (4 * jj + 4) // 2 - 1
                        nc.tensor.matmul(
                            zp[jj][:, off:off + w], ones8[:, :, :],
                            ek[:, :, off:off + w],
                            start=(pr == 0), stop=(pr == last),
                            perf_mode=DR, skip_group_check=True)
                        for e0 in range(2):
                            nc.tensor.matmul(
                                op[jj][e0][:, off:off + w],
                                vp8[pr][:, :, e0 * 128:(e0 + 1) * 128],
                                ek[:, :, off:off + w],
                                start=(pr == 0), stop=(pr == last),
                                perf_mode=DR, skip_group_check=True)

                    for pr in range(npair):
                        k0, k1 = 2 * pr, 2 * pr + 1
                        off0 = max(0, 128 * k0 - SC * j)
                        w0 = SC - off0
                        diag = 128 * k1 >= SC * j
                        ek = work.tile([128, 2, SC], F8, tag=f"exp{pr}",
                                       name=f"exp{pr}")
                        # both sub-blocks computed over [off0:]; the upper
                        # block's fully-masked strip gets real (tiny) scores
                        # and is zeroed by the combo mask below
                        for i, kk in enumerate((k0, k1)):
                            sp = psA.tile([128, SC], F32, tag="mm", name="mm")
                            nc.tensor.matmul(
                                sp[:, off0:],
                                x8[:, :, 128 * kk:128 * (kk + 1)],
                                tT8[:, :, SC * j + off0:SC * (j + 1)],
                                start=True, stop=True, perf_mode=DR)
                            nc.scalar.activation(
                                out=ek[:, i, off0:], in_=sp[:, off0:],
                                func=AF.Exp, scale=EXP_SCALE)
                            if diag:  # causal masks, gpsimd multiplies only
                                if i == 0:
                                    nc.gpsimd.tensor_mul(
                                        out=ek[:, 0, off0:off0 + 128],
                                        in0=ek[:, 0, off0:off0 + 128],
                                        in1=cmask8[:])
                                else:
                                    nc.gpsimd.tensor_mul(
                                        out=ek[:, 1, off0:off0 + 256],
                                        in0=ek[:, 1, off0:off0 + 256],
                                        in1=cmask28[:])
                        pend.append((j, pr, ek, off0, w0))
                        if len(pend) > 3:
                            emit_zav(pend.pop(0))
                        # stream the next head's tT/v groups into this head's
                        # attention so head boundaries carry no copy stall
                        it += 1
                        want = (len(pending) * it + n_iters - 1) // n_iters \
                            if pending else 0
                        while done < want:
                            pending[done]()
                            done += 1
                    for item in pend:
                        emit_zav(item)
                    # normalize: ont = oT * (1/Z'); Z' replicated over
                    # partitions; ones8=0.5 makes 1/Z' fold the fp8 scales
                    zb = work.tile([128, SC], F32, tag="zb", name="zb")
                    nc.vector.reciprocal_approx_fast(out=zb[:], in_=zp[j][:])
                    cols = slice(j * SC, (j + 1) * SC)
                    for e0 in range(2):
                        nc.vector.tensor_mul(
                            out=ont8[:, 2 * h + e0, cols], in0=op[j][e0][:],
                            in1=zb[:])
                while done < len(pending):
                    pending[done]()
                    done += 1
                if nxt:
                    cur = nxt

            # ---- wo projection + LN1 ----
            r1 = [acts.tile([128, S], F32, tag=f"r1_{d0}", name=f"r1_{d0}")
                  for d0 in range(ND)]
            r1b = [lnp.tile([128, S], BF16, tag=f"lnsrcb{d0}", name=f"r1b_{d0}")
                   for d0 in range(ND)]

            def ln_sq(j, srcb, w, j0, w0):
                cols = slice(j0, j0 + w)
                sq = [lnp.tile([128, w0], BF16, tag=f"lnsq{d0}_{j % 2}",
                               name=f"lnsq{d0}_{j}") for d0 in range(ND)]
                for d0 in range(ND):
                    nc.vector.tensor_mul(out=sq[d0][:, :w],
                                         in0=srcb[d0][:, cols],
                                         in1=srcb[d0][:, cols])
                return sq

            def ln_chain(w, c0, stats, src, gamma, beta, dst, dst8=None,
                         dma_out=None):
                """Row math + apply for cols [c0, c0+w) (DVE/ACT, no PE)."""
                cols = slice(c0, c0 + w)
                mup, m2p = stats
                musq = work.tile([128, w], F32, tag="musq", name="musq")
                nc.scalar.activation(out=musq[:], in_=mup[:, :w], func=AF.Square)
                var = work.tile([128, w], F32, tag="var", name="var")
                nc.vector.tensor_sub(out=var[:], in0=m2p[:, :w], in1=musq[:])
                sd = work.tile([128, w], F32, tag="sd", name="sd")
                nc.scalar.activation(out=sd[:], in_=var[:], func=AF.Sqrt,
                                     bias=eps_t[:])
                rstd = work.tile([128, w], F32, tag="rstd", name="rstd")
                nc.vector.reciprocal_approx_fast(out=rstd[:], in_=sd[:])
                mr = work.tile([128, w], F32, tag="mr", name="mr")
                nc.vector.tensor_mul(out=mr[:], in0=mup[:, :w], in1=rstd[:])
                for d0 in range(ND):
                    # out = src*(g*rstd) - (mr*g - b)
                    t = work.tile([128, w], F32, tag="lnt", name="lnt")
                    nc.vector.scalar_tensor_tensor(
                        out=t[:], in0=src[d0][:, cols], scalar=gamma[d0],
                        in1=rstd[:], op0=ALU.mult, op1=ALU.mult)
                    bb = work.tile([128, w], F32, tag="lnb", name="lnb")
                    nc.vector.tensor_scalar(
                        out=bb[:], in0=mr[:],
                        scalar1=gamma[d0], scalar2=beta[d0],
                        op0=ALU.mult, op1=ALU.subtract)
                    nc.vector.tensor_sub(out=dst[d0][:, cols],
                                         in0=t[:], in1=bb[:])
                    if dst8 is not None:  # fp8 twin (16x) for the next matmul
                        nc.vector.tensor_scalar_mul(
                            out=dst8[:, d0, cols], in0=dst[d0][:, cols],
                            scalar1=16.0)
                    if dma_out is not None:
                        nc.sync.dma_start(out=dma_out[d0][:, cols],
                                          in_=dst[d0][:, cols])

            # wo chunk j, then its stats matmuls right away (the DVE stt/sq
            # for chunk j runs under the wo matmuls of chunk j+1)
            x1 = [acts.tile([128, S], F32, tag=f"x1_{d0}", name=f"x1_{d0}")
                  for d0 in range(ND)]
            x1b8 = acts.tile([128, 2, S], F8, tag="x1b8", name="x1b8")
            st1 = []
            for j in range(NJ):
                cols = slice(j * SC, (j + 1) * SC)
                for d0 in range(ND):
                    pp = psA.tile([128, SC], F32, tag="mm", name="mm")
                    for c in range(4):
                        nc.tensor.matmul(
                            pp[:], wo8[:, 2 * c:2 * c + 2,
                                       d0 * 128:(d0 + 1) * 128],
                            ont8[:, 2 * c:2 * c + 2, cols],
                            start=(c == 0), stop=(c == 3), perf_mode=DR)
                    # r1 = psum*2^-12 + (x + wo_b)   (f32 + bf16 twin)
                    nc.vector.scalar_tensor_tensor(
                        out=r1b[d0][:, cols], in0=pp[:], scalar=sA[:],
                        in1=xres[d0][:, cols], op0=ALU.mult, op1=ALU.add)
                    nc.vector.scalar_tensor_tensor(
                        out=r1[d0][:, cols], in0=pp[:], scalar=sA[:],
                        in1=xres[d0][:, cols], op0=ALU.mult, op1=ALU.add)
                sq = ln_sq(j, r1b, SC, j * SC, SC)
                warm_pe(16)  # bridge the DVE stt/sq latency
                mup = psZ.tile([128, SC], F32, tag="z", name="z")
                for d0 in range(ND):
                    nc.tensor.matmul(mup[:], invd_bf[:], r1b[d0][:, cols],
                                     start=(d0 == 0), stop=(d0 == ND - 1))
                m2p = psO.tile([128, SC], F32, tag="o", name="m2p")
                for d0 in range(ND):
                    nc.tensor.matmul(m2p[:], invd_bf[:], sq[d0][:],
                                     start=(d0 == 0), stop=(d0 == ND - 1))
                st1.append((mup, m2p))
            # chains run on DVE/ACT while the PE idles over warm dummies
            warm_pe(44)
            for j in range(NJ):
                ln_chain(SC, j * SC, st1[j], r1, ln1_g, ln1_b, x1,
                         dst8=x1b8)

            # ---- FFN ----
            hT8 = acts.tile([128, NF, S], F8, tag="hT8", name="hT8")
            for j in range(NJ):
                cols = slice(j * SC, (j + 1) * SC)
                for f0 in range(NF):
                    fp = psA.tile([128, SC], F32, tag="mm", name="mm")
                    nc.tensor.matmul(
                        fp[:], ff18[:, :, f0 * 128:(f0 + 1) * 128],
                        x1b8[:, :, cols], start=True, stop=True, perf_mode=DR)
                    # hT8 = 16*relu(x1@ff1 + b) fused on ACT
                    nc.scalar.activation(out=hT8[:, f0, cols], in_=fp[:],
                                         func=AF.Relu, scale=2.0 ** -6,
                                         bias=ff1b16[f0])
                if j == 0:
                    warm_pe(8)  # bridge chain(1) finishing on DVE

            # ff2 + LN2, chunked with the last chunks narrow so the final
            # chain after the last matmul is short. Stats matmuls for chunk c
            # are emitted after the ff2 matmuls of chunk c+1 (PE never waits
            # on the DVE residual adds), and chain c follows its stats.
            CH = [(0, 256), (256, 256), (512, 256), (768, 128), (896, 128)]
            r2 = [acts.tile([128, S], F32, tag=f"r2_{d0}", name=f"r2_{d0}")
                  for d0 in range(ND)]
            r2b = [lnp.tile([128, S], BF16, tag=f"lnsrcb{d0}", name=f"r2b_{d0}")
                   for d0 in range(ND)]
            outT = [acts.tile([128, S], F32, tag=f"out{d0}", name=f"out{d0}")
                    for d0 in range(ND)]
            dmao = [out_d[d0] for d0 in range(ND)]

            def ff2_chunk(c0, w):
                cols = slice(c0, c0 + w)
                for d0 in range(ND):
                    fp = psA.tile([128, SC], F32, tag="mm", name="mm")
                    for pr in range(NF // 2):
                        nc.tensor.matmul(
                            fp[:, :w],
                            ff28[:, 2 * pr:2 * pr + 2, d0 * 128:(d0 + 1) * 128],
                            hT8[:, 2 * pr:2 * pr + 2, cols],
                            start=(pr == 0), stop=(pr == NF // 2 - 1),
                            perf_mode=DR)
                    # r2 = psum*2^-10 + ff2_b + x1  (f32 + bf16 twin)
                    tmp = work.tile([128, w], F32, tag="f2t", name="f2t")
                    nc.vector.tensor_scalar(
                        out=tmp[:], in0=fp[:, :w], scalar1=sB[:],
                        scalar2=ff2_b[d0], op0=ALU.mult, op1=ALU.add)
                    nc.vector.tensor_add(out=r2b[d0][:, cols], in0=tmp[:],
                                         in1=x1[d0][:, cols])
                    nc.vector.tensor_add(out=r2[d0][:, cols], in0=tmp[:],
                                         in1=x1[d0][:, cols])

            def ln2_stats(i, c0, w):
                sq2 = ln_sq(i, r2b, w, c0, 256)
                mup = psZ.tile([128, SC], F32, tag="z", name="z")
                for d0 in range(ND):
                    nc.tensor.matmul(mup[:, :w], invd_bf[:],
                                     r2b[d0][:, c0:c0 + w],
                                     start=(d0 == 0), stop=(d0 == ND - 1))
                m2p = psO.tile([128, SC], F32, tag="o", name="m2p")
                for d0 in range(ND):
                    nc.tensor.matmul(m2p[:, :w], invd_bf[:], sq2[d0][:, :w],
                                     start=(d0 == 0), stop=(d0 == ND - 1))
                return (mup, m2p)

            prev = None
            for i, (c0, w) in enumerate(CH):
                ff2_chunk(c0, w)
                if prev is not None:
                    pc0, pw = CH[i - 1]
                    st = ln2_stats(i - 1, pc0, pw)
                    ln_chain(pw, pc0, st, r2, ln2_g, ln2_b, outT,
                             dma_out=dmao)
                prev = i
            c0, w = CH[-1]
            st = ln2_stats(len(CH) - 1, c0, w)
            ln_chain(w, c0, st, r2, ln2_g, ln2_b, outT, dma_out=dmao)

    nc.compile()
    return nc


def _np_reference(x, attention_mask, wq, wk, wv, wo_w, wo_b, ln1_g, ln1_b,
                  ff1_w, ff1_b, ff2_w, ff2_b, ln2_g, ln2_b):
    """Numpy fallback (only used if attention_mask has zeros)."""
    def ln(t, g, b):
        mu = t.mean(-1, keepdims=True)
        var = t.var(-1, keepdims=True)
        return (t - mu) / np.sqrt(var + LN_EPS) * g + b
    Bn, Sn, Dn = x.shape
    q = np.einsum('bsd,hed->bhse', x, wq)
    k = np.einsum('bsd,hed->bhse', x, wk)
    v = np.einsum('bsd,hed->bhse', x, wv)
    sc = np.einsum('bhse,bhte->bhst', q, k) / np.sqrt(np.float32(Dn))
    idx = np.arange(Sn)
    causal = idx[None, :] > idx[:, None]
    m = attention_mask.astype(bool)
    valid = m[:, None, :] & m[:, :, None]
    cond = causal[None] | ~valid
    sc = np.where(cond[:, None], -np.inf, sc)
    sc = sc - np.nanmax(np.where(np.isinf(sc), np.nan, sc), axis=-1,
                        keepdims=True)
    e = np.exp(sc)
    e = np.where(np.isnan(e), 0.0, e)
    att = e / np.maximum(e.sum(-1, keepdims=True), 1e-30)
    ho = np.einsum('bhst,bhte->bhse', att, v)
    cat = np.transpose(ho, (0, 2, 1, 3)).reshape(Bn, Sn, -1)
    mh = cat @ wo_w.T + wo_b
    x1 = ln(x + mh, ln1_g, ln1_b)
    hh = np.maximum(x1 @ ff1_w.T + ff1_b, 0.0)
    ff = hh @ ff2_w.T + ff2_b
    return ln(x1 + ff, ln2_g, ln2_b).astype(np.float32)


def _f8(a):
    return np.clip(a, -240.0, 240.0).astype(ml_dtypes.float8_e4m3)


def _prep_inputs(inputs):
    x = np.asarray(inputs["x"], np.float32)
    wq = np.asarray(inputs["wq"], np.float32)
    wk = np.asarray(inputs["wk"], np.float32)
    wv = np.asarray(inputs["wv"], np.float32)
    wo_w = np.asarray(inputs["wo_w"], np.float32)
    ff1_w = np.asarray(inputs["ff1_w"], np.float32)
    ff2_w = np.asarray(inputs["ff2_w"], np.float32)

    # A[h] = wq[h]^T wk[h] so scores = x A x^T (q/k projections fused away)
    A = np.einsum('hed,hef->hdf', wq, wk)
    A8 = np.zeros((128, 2, H * D), np.float32)
    for h in range(H):
        A8[:, :, h * D:(h + 1) * D] = (2048.0 * A[h]).reshape(
            2, 128, D).transpose(1, 0, 2)
    wvT = np.ascontiguousarray(wv.transpose(2, 0, 1).reshape(D, HE))
    wv8 = (32.0 * wvT).reshape(2, 128, HE).transpose(1, 0, 2)
    wo8 = (64.0 * wo_w.T).reshape(NF, 128, D).transpose(1, 0, 2)
    ff18 = (64.0 * ff1_w.T).reshape(2, 128, FF).transpose(1, 0, 2)
    ff28 = (64.0 * ff2_w.T).reshape(NF, 128, D).transpose(1, 0, 2)

    biasp = np.zeros((128, 18), np.float32)
    biasp[:, 0:8] = (16.0 * np.asarray(inputs["ff1_b"], np.float32)
                     ).reshape(8, 128).T
    biasp[:, 8:10] = np.asarray(inputs["ff2_b"], np.float32).reshape(2, 128).T
    biasp[:, 10:12] = np.asarray(inputs["ln1_g"], np.float32).reshape(2, 128).T
    biasp[:, 12:14] = np.asarray(inputs["ln1_b"], np.float32).reshape(2, 128).T
    biasp[:, 14:16] = np.asarray(inputs["ln2_g"], np.float32).reshape(2, 128).T
    biasp[:, 16:18] = np.asarray(inputs["ln2_b"], np.float32).reshape(2, 128).T

    shared = dict(
        A8=_f8(A8), wv8=_f8(wv8), wo8=_f8(wo8), ff18=_f8(ff18),
        ff28=_f8(ff28), biasp=biasp,
    )
    wo_b = np.asarray(inputs["wo_b"], np.float32)
    in_maps = []
    for b in range(B):
        xT = np.ascontiguousarray(x[b].T)  # [D, S]
        m = dict(shared)
        m["x8"] = _f8((16.0 * xT).reshape(2, 128, S).transpose(1, 0, 2))
        m["xres"] = (xT + wo_b[:, None]).reshape(ND, 128, S)
        in_maps.append(m)
    return in_maps


def run_sharded(inputs, trace=False, trace_kwargs=None):
    if "nc" not in _CACHE:
        _CACHE["nc"] = _build()
    nc = _CACHE["nc"]
    in_maps = _prep_inputs(inputs)
    res = run_bass_kernel_spmd(nc, in_maps, list(range(N_CORES)), trace=trace,
                               **(trace_kwargs or {}))
    outs = []
    for b in range(B):
        r = np.asarray(res.results[b]["out"], np.float32).reshape(D, S)
        outs.append(r.T)
    return np.stack(outs), res


def kernel(**inputs) -> np.ndarray:
    mask = np.asarray(inputs["attention_mask"])
    if not np.all(mask != 0):
        return _np_reference(**{k: np.asarray(v) for k, v in inputs.items()})
    out, _ = run_sharded(inputs, trace=False)
    return out


# revision 25
# speedup vs baseline: 1.1008x; 1.1008x over previous
"""Trainium2 Bass kernel for nn_DecoderBlock (B=8, S=1024, D=256, H=4 heads
of full width 256, FF=1024).

Strategy: pure data parallelism — B=8 batch elements across 8 NeuronCores,
zero collectives. Per core, one full decoder block in "transposed" activation
layout (features on SBUF partitions, tokens on the free dim), with the heavy
matmuls in fp8e4 DoubleRow mode (2 fp8 MACs per PE cell per cycle):

  tT   = (A8)^T @ x8        per head, A = wq^T wk precomputed on host, so
                            scores = x A x^T needs no separate q/k projections
  v    = (x8)^T @ wv8       [tokens, E], stored as fp8 key-block pairs
  sp   = (x8_keys)^T @ tT   causal lower-triangle blocks only
  ek   = exp(sp * 2^-15)    fp8, causal triangle masked on GpSimd
  Z    = halves^T @ ek      softmax denominators (DoubleRow "ones" = 0.5 so
                            the reciprocal also fixes the v/ont scale)
  oT   = v^T @ ek ; ont = oT * (1/Z)  -> fp8 * 64
  mh   = wo8^T @ ont ; r1 = mh*2^-12 + (x^T + wo_b) ; LN1
  ffn  : h = relu(ff18^T @ x1b8 * 2^-10 + 16*ff1_b) fp8 *16
         r2 = ff28^T @ h * 2^-10 + ff2_b + x1 ; LN2 in 4 chunks of 256 so the
         final LayerNorm tail after the last matmul is short.

All fp8 operands carry power-of-2 scales (x*16, A*4096, w*64, ...) folded
into activation scales / host-prescaled biases; accumulation is fp32 in PSUM
and LayerNorm math is fp32 (stats matmuls bf16). The attention_mask input is
all ones per the problem spec (causal mask only); if a mask with zeros ever
shows up, we fall back to a numpy reference.
"""

import numpy as np
import ml_dtypes

import concourse.bass as bass
import concourse.mybir as mybir
import concourse.tile as tile
from concourse import bacc
from concourse.bass_utils import run_bass_kernel_spmd

F32 = mybir.dt.float32
BF16 = mybir.dt.bfloat16
F8 = mybir.dt.float8e4
AF = mybir.ActivationFunctionType
ALU = mybir.AluOpType
DR = mybir.MatmulPerfMode.DoubleRow

N_CORES = 8
B, S, D, H, E, HE, FF = 8, 1024, 256, 4, 256, 1024, 1024
SC = 512          # token (free-dim) chunk for attention / wo / ff1
NJ = S // SC      # 2 chunks
SC2 = 256         # token chunk for ff2 + LN2 (short tail)
NJ2 = S // SC2    # 4 chunks
ND = D // 128     # 2 partition chunks of features
NF = FF // 128    # 8 partition chunks of ff features
LN_EPS = 1e-5
EXP_SCALE = 2.0 ** -14  # scores psum = (16x)·(64t) = 1024·s ; softmax /16

_CACHE = {}


def _build():
    nc = bacc.Bacc("TRN2", target_bir_lowering=False, debug=False,
                   num_devices=N_CORES)

    # ---- DRAM parameters (per-core shard + replicated weights) ----
    x8_d = nc.dram_tensor("x8", [128, 2, S], F8, kind="ExternalInput")
    A8_d = nc.dram_tensor("A8", [128, 2, H * D], F8, kind="ExternalInput")
    wv8_d = nc.dram_tensor("wv8", [128, 2, HE], F8, kind="ExternalInput")
    bias_d = nc.dram_tensor("biasp", [128, 18], F32, kind="ExternalInput")
    wo8_d = nc.dram_tensor("wo8", [128, NF, D], F8, kind="ExternalInput")
    ff18_d = nc.dram_tensor("ff18", [128, 2, FF], F8, kind="ExternalInput")
    ff28_d = nc.dram_tensor("ff28", [128, NF, D], F8, kind="ExternalInput")
    xres_d = nc.dram_tensor("xres", [ND, 128, S], BF16, kind="ExternalInput")
    out_d = nc.dram_tensor("out", [ND, 128, S], F32, kind="ExternalOutput")

    with tile.TileContext(nc) as tc:
        with tc.tile_pool(name="consts", bufs=1) as consts, \
             tc.tile_pool(name="acts", bufs=1) as acts, \
             tc.tile_pool(name="work", bufs=2) as work, \
             tc.tile_pool(name="lnp", bufs=1) as lnp, \
             tc.tile_pool(name="psA", bufs=3, space="PSUM") as psA, \
             tc.tile_pool(name="psO", bufs=3, space="PSUM") as psO, \
             tc.tile_pool(name="psZ", bufs=2, space="PSUM") as psZ:

            def loadc(dram, shape, dt):
                t = consts.tile(shape, dt, tag=dram.name, name=dram.name)
                nc.sync.dma_start(out=t[:], in_=dram[:])
                return t

            # DMA in first-use order so the PE can start ASAP
            x8 = loadc(x8_d, [128, 2, S], F8)
            A8 = loadc(A8_d, [128, 2, H * D], F8)
            wv8 = loadc(wv8_d, [128, 2, HE], F8)
            biasp = loadc(bias_d, [128, 18], F32)
            wo8 = loadc(wo8_d, [128, NF, D], F8)
            ff18 = loadc(ff18_d, [128, 2, FF], F8)
            ff28 = loadc(ff28_d, [128, NF, D], F8)
            xres = []
            for i in range(ND):
                t = consts.tile([128, S], BF16, tag=f"xres{i}", name=f"xres{i}")
                nc.sync.dma_start(out=t[:], in_=xres_d[i])
                xres.append(t)

            # bias pack columns
            ff1b16 = [biasp[:, f0:f0 + 1] for f0 in range(NF)]
            ff2_b = [biasp[:, 8 + i:9 + i] for i in range(ND)]
            ln1_g16 = [biasp[:, 10 + i:11 + i] for i in range(ND)]  # 16*g
            ln1_b16 = [biasp[:, 12 + i:13 + i] for i in range(ND)]  # 16*b
            ln2_g = [biasp[:, 14 + i:15 + i] for i in range(ND)]
            ln2_b = [biasp[:, 16 + i:17 + i] for i in range(ND)]

            # "halves" matrix for the Z matmul: DoubleRow over key-block
            # pairs with value 0.5, so Z' = Z/2 and 1/Z' folds the v(32x) and
            # ont(64x) scales into the existing reciprocal. [128,128] result
            # is replicated on all partitions.
            ones8 = consts.tile([128, 2, 128], F8, tag="ones8", name="ones8")
            nc.vector.memset(ones8[:], 0.5)
            onesb = consts.tile([128, 128], BF16, tag="onesb", name="onesb")
            nc.vector.memset(onesb[:], 1.0)
            invd_bf = consts.tile([128, 128], BF16, tag="invd", name="invd")
            nc.vector.memset(invd_bf[:], 1.0 / D)  # 2^-8, exact in bf16
            eps_t = consts.tile([128, 1], F32, tag="eps", name="eps")
            nc.vector.memset(eps_t[:], LN_EPS)
            # multiplicative causal mask for the diagonal 128x128 block of a
            # transposed [t, s] exp tile: 1 where t <= s else 0
            cmaskf = consts.tile([128, 128], F32, tag="cmaskf", name="cmaskf")
            nc.gpsimd.memset(cmaskf[:], 1.0)
            nc.gpsimd.affine_select(
                out=cmaskf[:], in_=cmaskf[:],
                compare_op=ALU.is_ge, fill=0.0,
                base=0, pattern=[[1, 128]], channel_multiplier=-1,
            )
            cmask8 = consts.tile([128, 128], F8, tag="cmask8", name="cmask8")
            nc.gpsimd.tensor_copy(out=cmask8[:], in_=cmaskf[:])
            # [zeros | triangle] combo for the upper block of a diagonal
            # pair: zeroes the fully-masked strip and masks the diagonal in
            # one gpsimd multiply (op-type switches cost library reloads)
            cmask28 = consts.tile([128, 256], F8, tag="cmask28", name="cmask28")
            nc.gpsimd.memset(cmask28[:, 0:128], 0.0)
            nc.gpsimd.tensor_copy(out=cmask28[:, 128:256], in_=cmaskf[:])
            # bf16 scaled identity matrices: fold the residual adds into the
            # wo / ff2 PSUM accumulations (diag*4096 x xres ; diag*64 x x1bb)
            idf = consts.tile([128, 128], F32, tag="idf", name="idf")
            nc.gpsimd.memset(idf[:], 1.0)
            nc.gpsimd.affine_select(
                out=idf[:], in_=idf[:], compare_op=ALU.is_ge, fill=0.0,
                base=0, pattern=[[1, 128]], channel_multiplier=-1)
            nc.gpsimd.affine_select(
                out=idf[:], in_=idf[:], compare_op=ALU.is_ge, fill=0.0,
                base=0, pattern=[[-1, 128]], channel_multiplier=1)
            id4096 = consts.tile([128, 128], BF16, tag="id4096", name="id4096")
            nc.gpsimd.tensor_scalar_mul(out=id4096[:], in0=idf[:],
                                        scalar1=4096.0)
            id64 = consts.tile([128, 128], BF16, tag="id64", name="id64")
            nc.gpsimd.tensor_scalar_mul(out=id64[:], in0=idf[:], scalar1=64.0)

            # PE warmup: dummy matmuls (no DMA dependency) keep the HAM
            # clock-gate at 8/8 across known PE-idle bubbles.
            def warm_pe(n):
                for _ in range(n):
                    wp = psA.tile([128, 128], F32, tag="mm", name="warm")
                    nc.tensor.matmul(wp[:], onesb[:], onesb[:],
                                     start=True, stop=True)

            warm_pe(26)

            # ---- attention: per head -> ont8 [128, 2H, S] fp8 (64*o) ----
            ont8 = acts.tile([128, 2 * H, S], F8, tag="ont8", name="ont8")

            def make_head(h):
                """Allocate head h's tT/v tiles and return thunks that emit
                one projection matmul-group each, in first-use order."""
                tT8 = work.tile([128, 2, S], F8, tag="tT", name=f"tT{h}")
                vp8 = [work.tile([128, 2, E], F8, tag=f"vp{p}",
                                 name=f"vp{h}_{p}") for p in range(4)]

                def t_group(e0, j):
                    cols = slice(j * SC, (j + 1) * SC)
                    p = psA.tile([128, SC], F32, tag="mm", name="mm")
                    nc.tensor.matmul(
                        p[:], A8[:, :, h * D + e0 * 128: h * D + (e0 + 1) * 128],
                        x8[:, :, cols], start=True, stop=True, perf_mode=DR)
                    nc.scalar.activation(out=tT8[:, e0, cols], in_=p[:],
                                         func=AF.Copy, scale=2.0 ** -9)

                def v_group(pr):
                    p = psA.tile([128, 2, E], F32, tag="mm", name="mm")
                    for i in range(2):
                        t0 = 2 * pr + i
                        nc.tensor.matmul(
                            p[:, i, :], x8[:, :, t0 * 128:(t0 + 1) * 128],
                            wv8[:, :, h * E:(h + 1) * E],
                            start=True, stop=True, perf_mode=DR)
                    nc.vector.tensor_scalar_mul(
                        out=vp8[pr][:, :, :], in0=p[:, :, :],
                        scalar1=1.0 / 16.0)

                thunks = [lambda e0=0: t_group(0, 0), lambda: v_group(0),
                          lambda e0=1: t_group(1, 0), lambda: v_group(1),
                          lambda: t_group(0, 1), lambda: v_group(2),
                          lambda: t_group(1, 1), lambda: v_group(3)]
                return tT8, vp8, thunks

            cur = make_head(0)
            for t in cur[2]:
                t()

            for h in range(H):
                tT8, vp8, _ = cur
                nxt = make_head(h + 1) if h + 1 < H else None
                pending = list(nxt[2]) if nxt else []
                n_iters = 6  # total pair iterations this head
                it = 0
                done = 0
                zp = [psZ.tile([128, SC], F32, tag="z", name="z")
                      for j in range(NJ)]
                op = [[psO.tile([128, SC], F32, tag="o", name="o")
                       for _ in range(2)] for j in range(NJ)]
                for j in range(NJ):
                    npair = (4 * j + 4) // 2
                    pend = []

                    def emit_zav(item):
                        jj, pr, ek, off, w = item
                        last = (4 * jj + 4) // 2 - 1
                        nc.tensor.matmul(
                            zp[jj][:, off:off + w], ones8[:, :, :],
                            ek[:, :, off:off + w],
                            start=(pr == 0), stop=(pr == last),
                            perf_mode=DR, skip_group_check=True)
                        for e0 in range(2):
                            nc.tensor.matmul(
                                op[jj][e0][:, off:off + w],
                                vp8[pr][:, :, e0 * 128:(e0 + 1) * 128],
                                ek[:, :, off:off + w],
                                start=(pr == 0), stop=(pr == last),
                                perf_mode=DR, skip_group_check=True)

                    for pr in range(npair):
                        k0, k1 = 2 * pr, 2 * pr + 1
                        off0 = max(0, 128 * k0 - SC * j)
                        w0 = SC - off0
                        diag = 128 * k1 >= SC * j
                        ek = work.tile([128, 2, SC], F8, tag=f"exp{pr}",
                                       name=f"exp{pr}")
                        # both sub-blocks computed over [off0:]; the upper
                        # block's fully-masked strip gets real (tiny) scores
                        # and is zeroed by the combo mask below
                        for i, kk in enumerate((k0, k1)):
                            sp = psA.tile([128, SC], F32, tag="mm", name="mm")
                            nc.tensor.matmul(
                                sp[:, off0:],
                                x8[:, :, 128 * kk:128 * (kk + 1)],
                                tT8[:, :, SC * j + off0:SC * (j + 1)],
                                start=True, stop=True, perf_mode=DR)
                            nc.scalar.activation(
                                out=ek[:, i, off0:], in_=sp[:, off0:],
                                func=AF.Exp, scale=EXP_SCALE)
                            if diag:  # causal masks, gpsimd multiplies only
                                if i == 0:
                                    nc.gpsimd.tensor_mul(
                                        out=ek[:, 0, off0:off0 + 128],
                                        in0=ek[:, 0, off0:off0 + 128],
                                        in1=cmask8[:])
                                else:
                                    nc.gpsimd.tensor_mul(
                                        out=ek[:, 1, off0:off0 + 256],
                                        in0=ek[:, 1, off0:off0 + 256],
                                        in1=cmask28[:])
                        pend.append((j, pr, ek, off0, w0))
                        if len(pend) > 3:
                            emit_zav(pend.pop(0))
                        # stream the next head's tT/v groups into this head's
                        # attention so head boundaries carry no copy stall
                        it += 1
                        want = (len(pending) * it + n_iters - 1) // n_iters \
                            if pending else 0
                        while done < want:
                            pending[done]()
                            done += 1
                    for item in pend:
                        emit_zav(item)
                    # normalize: ont = oT * (1/Z'); Z' replicated over
                    # partitions; ones8=0.5 makes 1/Z' fold the fp8 scales
                    zb = work.tile([128, SC], F32, tag="zb", name="zb")
                    nc.vector.reciprocal_approx_fast(out=zb[:], in_=zp[j][:])
                    cols = slice(j * SC, (j + 1) * SC)
                    for e0 in range(2):
                        nc.vector.tensor_mul(
                            out=ont8[:, 2 * h + e0, cols], in0=op[j][e0][:],
                            in1=zb[:])
                while done < len(pending):
                    pending[done]()
                    done += 1
                if nxt:
                    cur = nxt

            # ---- wo projection + LN1 ----
            r1b = [lnp.tile([128, S], BF16, tag=f"lnsrcb{d0}", name=f"r1b_{d0}")
                   for d0 in range(ND)]

            def ln_sq(j, srcb, w, j0, w0):
                cols = slice(j0, j0 + w)
                sq = [lnp.tile([128, w0], BF16, tag=f"lnsq{d0}_{j % 2}",
                               name=f"lnsq{d0}_{j}") for d0 in range(ND)]
                for d0 in range(ND):
                    nc.vector.tensor_mul(out=sq[d0][:, :w],
                                         in0=srcb[d0][:, cols],
                                         in1=srcb[d0][:, cols])
                return sq

            def ln_chain(w, c0, stats, src, gamma, beta, dst, dst8=None,
                         dma_out=None):
                """Row math + apply for cols [c0, c0+w) (DVE/ACT, no PE)."""
                cols = slice(c0, c0 + w)
                mup, m2p = stats
                musq = work.tile([128, w], F32, tag="musq", name="musq")
                nc.scalar.activation(out=musq[:], in_=mup[:, :w], func=AF.Square)
                var = work.tile([128, w], F32, tag="var", name="var")
                nc.vector.tensor_sub(out=var[:], in0=m2p[:, :w], in1=musq[:])
                sd = work.tile([128, w], F32, tag="sd", name="sd")
                nc.scalar.activation(out=sd[:], in_=var[:], func=AF.Sqrt,
                                     bias=eps_t[:])
                rstd = work.tile([128, w], F32, tag="rstd", name="rstd")
                nc.vector.reciprocal_approx_fast(out=rstd[:], in_=sd[:])
                mr = work.tile([128, w], F32, tag="mr", name="mr")
                nc.vector.tensor_mul(out=mr[:], in0=mup[:, :w], in1=rstd[:])
                for d0 in range(ND):
                    # out = src*(g*rstd) - (mr*g - b)
                    t = work.tile([128, w], F32, tag="lnt", name="lnt")
                    nc.vector.scalar_tensor_tensor(
                        out=t[:], in0=src[d0][:, cols], scalar=gamma[d0],
                        in1=rstd[:], op0=ALU.mult, op1=ALU.mult)
                    bb = work.tile([128, w], F32, tag="lnb", name="lnb")
                    nc.vector.tensor_scalar(
                        out=bb[:], in0=mr[:],
                        scalar1=gamma[d0], scalar2=beta[d0],
                        op0=ALU.mult, op1=ALU.subtract)
                    nc.vector.tensor_sub(out=dst[d0][:, cols],
                                         in0=t[:], in1=bb[:])
                    if dst8 is not None:  # fp8 cast for the next matmul
                        nc.vector.tensor_copy(out=dst8[:, d0, cols],
                                              in_=dst[d0][:, cols])
                    if dma_out is not None:
                        nc.sync.dma_start(out=dma_out[d0][:, cols],
                                          in_=dst[d0][:, cols])

            # wo chunk j (residual folded in via the scaled identity), then
            # its stats matmuls right away; chain j produces the bf16
            # carrier x1bb = 16*x1 plus its fp8 cast for the ff1 matmul.
            x1bb = [acts.tile([128, S], BF16, tag=f"x1_{d0}", name=f"x1_{d0}")
                    for d0 in range(ND)]
            x1b8 = acts.tile([128, 2, S], F8, tag="x1b8", name="x1b8")
            st1 = []
            for j in range(NJ):
                cols = slice(j * SC, (j + 1) * SC)
                for d0 in range(ND):
                    pp = psA.tile([128, SC], F32, tag="mm", name="mm")
                    for c in range(4):
                        nc.tensor.matmul(
                            pp[:], wo8[:, 2 * c:2 * c + 2,
                                       d0 * 128:(d0 + 1) * 128],
                            ont8[:, 2 * c:2 * c + 2, cols],
                            start=(c == 0), stop=False, perf_mode=DR)
                    nc.tensor.matmul(pp[:], id4096[:], xres[d0][:, cols],
                                     start=False, stop=True)
                    # r1 = psum*2^-12 = mh + x + wo_b   (bf16)
                    nc.vector.tensor_scalar_mul(
                        out=r1b[d0][:, cols], in0=pp[:], scalar1=2.0 ** -12)
                sq = ln_sq(j, r1b, SC, j * SC, SC)
                warm_pe(10)  # bridge the DVE latency
                mup = psZ.tile([128, SC], F32, tag="z", name="z")
                for d0 in range(ND):
                    nc.tensor.matmul(mup[:], invd_bf[:], r1b[d0][:, cols],
                                     start=(d0 == 0), stop=(d0 == ND - 1))
                m2p = psO.tile([128, SC], F32, tag="o", name="m2p")
                for d0 in range(ND):
                    nc.tensor.matmul(m2p[:], invd_bf[:], sq[d0][:],
                                     start=(d0 == 0), stop=(d0 == ND - 1))
                st1.append((mup, m2p))
            # chains run on DVE/ACT while the PE idles over warm dummies
            warm_pe(36)
            for j in range(NJ):
                ln_chain(SC, j * SC, st1[j], r1b, ln1_g16, ln1_b16, x1bb,
                         dst8=x1b8)

            # ---- FFN ----
            hT8 = acts.tile([128, NF, S], F8, tag="hT8", name="hT8")
            for j in range(NJ):
                cols = slice(j * SC, (j + 1) * SC)
                for f0 in range(NF):
                    fp = psA.tile([128, SC], F32, tag="mm", name="mm")
                    nc.tensor.matmul(
                        fp[:], ff18[:, :, f0 * 128:(f0 + 1) * 128],
                        x1b8[:, :, cols], start=True, stop=True, perf_mode=DR)
                    # hT8 = 16*relu(x1@ff1 + b) fused on ACT
                    nc.scalar.activation(out=hT8[:, f0, cols], in_=fp[:],
                                         func=AF.Relu, scale=2.0 ** -6,
                                         bias=ff1b16[f0])
                if j == 0:
                    warm_pe(8)  # bridge chain(1) finishing on DVE

            # ff2 + LN2 in 4 chunks of 256 (x1 residual folded into the
            # PSUM): chain for chunk c hides under chunk c+1's matmuls.
            r2b = [lnp.tile([128, S], BF16, tag=f"lnsrcb{d0}", name=f"r2b_{d0}")
                   for d0 in range(ND)]
            outT = [acts.tile([128, S], F32, tag=f"out{d0}", name=f"out{d0}")
                    for d0 in range(ND)]
            dmao = [out_d[d0] for d0 in range(ND)]
            prev = None
            for j2 in range(NJ2):
                c0 = j2 * SC2
                cols = slice(c0, c0 + SC2)
                for d0 in range(ND):
                    fp = psA.tile([128, SC], F32, tag="mm", name="mm")
                    for pr in range(NF // 2):
                        nc.tensor.matmul(
                            fp[:, :SC2],
                            ff28[:, 2 * pr:2 * pr + 2, d0 * 128:(d0 + 1) * 128],
                            hT8[:, 2 * pr:2 * pr + 2, cols],
                            start=(pr == 0), stop=False, perf_mode=DR)
                    nc.tensor.matmul(fp[:, :SC2], id64[:], x1bb[d0][:, cols],
                                     start=False, stop=True)
                    # r2 = psum*2^-10 + ff2_b = ff + x1 + ff2_b  (bf16)
                    nc.vector.tensor_scalar(
                        out=r2b[d0][:, cols], in0=fp[:, :SC2],
                        scalar1=2.0 ** -10, scalar2=ff2_b[d0],
                        op0=ALU.mult, op1=ALU.add)
                sq2 = ln_sq(j2, r2b, SC2, c0, SC2)
                mup = psZ.tile([128, SC], F32, tag="z", name="z")
                for d0 in range(ND):
                    nc.tensor.matmul(mup[:, :SC2], invd_bf[:],
                                     r2b[d0][:, cols],
                                     start=(d0 == 0), stop=(d0 == ND - 1))
                m2p = psO.tile([128, SC], F32, tag="o", name="m2p")
                for d0 in range(ND):
                    nc.tensor.matmul(m2p[:, :SC2], invd_bf[:], sq2[d0][:, :SC2],
                                     start=(d0 == 0), stop=(d0 == ND - 1))
                if prev is not None:
                    ln_chain(SC2, prev[0], prev[1], r2b, ln2_g, ln2_b, outT,
                             dma_out=dmao)
                prev = (c0, (mup, m2p))
            ln_chain(SC2, prev[0], prev[1], r2b, ln2_g, ln2_b, outT,
                     dma_out=dmao)

    nc.compile()
    return nc


def _np_reference(x, attention_mask, wq, wk, wv, wo_w, wo_b, ln1_g, ln1_b,
                  ff1_w, ff1_b, ff2_w, ff2_b, ln2_g, ln2_b):
    """Numpy fallback (only used if attention_mask has zeros)."""
    def ln(t, g, b):
        mu = t.mean(-1, keepdims=True)
        var = t.var(-1, keepdims=True)
        return (t - mu) / np.sqrt(var + LN_EPS) * g + b
    Bn, Sn, Dn = x.shape
    q = np.einsum('bsd,hed->bhse', x, wq)
    k = np.einsum('bsd,hed->bhse', x, wk)
    v = np.einsum('bsd,hed->bhse', x, wv)
    sc = np.einsum('bhse,bhte->bhst', q, k) / np.sqrt(np.float32(Dn))
    idx = np.arange(Sn)
    causal = idx[None, :] > idx[:, None]
    m = attention_mask.astype(bool)
    valid = m[:, None, :] & m[:, :, None]
    cond = causal[None] | ~valid
    sc = np.where(cond[:, None], -np.inf, sc)
    sc = sc - np.nanmax(np.where(np.isinf(sc), np.nan, sc), axis=-1,
                        keepdims=True)
    e = np.exp(sc)
    e = np.where(np.isnan(e), 0.0, e)
    att = e / np.maximum(e.sum(-1, keepdims=True), 1e-30)
    ho = np.einsum('bhst,bhte->bhse', att, v)
    cat = np.transpose(ho, (0, 2, 1, 3)).reshape(Bn, Sn, -1)
    mh = cat @ wo_w.T + wo_b
    x1 = ln(x + mh, ln1_g, ln1_b)
    hh = np.maximum(x1 @ ff1_w.T + ff1_b, 0.0)
    ff = hh @ ff2_w.T + ff2_b
    return ln(x1 + ff, ln2_g, ln2_b).astype(np.float32)


def _f8(a):
    return np.clip(a, -240.0, 240.0).astype(ml_dtypes.float8_e4m3)


def _prep_inputs(inputs):
    x = np.asarray(inputs["x"], np.float32)
    wq = np.asarray(inputs["wq"], np.float32)
    wk = np.asarray(inputs["wk"], np.float32)
    wv = np.asarray(inputs["wv"], np.float32)
    wo_w = np.asarray(inputs["wo_w"], np.float32)
    ff1_w = np.asarray(inputs["ff1_w"], np.float32)
    ff2_w = np.asarray(inputs["ff2_w"], np.float32)

    # A[h] = wq[h]^T wk[h] so scores = x A x^T (q/k projections fused away)
    A = np.einsum('hed,hef->hdf', wq, wk)
    A8 = np.zeros((128, 2, H * D), np.float32)
    for h in range(H):
        A8[:, :, h * D:(h + 1) * D] = (2048.0 * A[h]).reshape(
            2, 128, D).transpose(1, 0, 2)
    wvT = np.ascontiguousarray(wv.transpose(2, 0, 1).reshape(D, HE))
    wv8 = (32.0 * wvT).reshape(2, 128, HE).transpose(1, 0, 2)
    wo8 = (64.0 * wo_w.T).reshape(NF, 128, D).transpose(1, 0, 2)
    ff18 = (64.0 * ff1_w.T).reshape(2, 128, FF).transpose(1, 0, 2)
    ff28 = (64.0 * ff2_w.T).reshape(NF, 128, D).transpose(1, 0, 2)

    biasp = np.zeros((128, 18), np.float32)
    biasp[:, 0:8] = (16.0 * np.asarray(inputs["ff1_b"], np.float32)
                     ).reshape(8, 128).T
    biasp[:, 8:10] = np.asarray(inputs["ff2_b"], np.float32).reshape(2, 128).T
    biasp[:, 10:12] = (16.0 * np.asarray(inputs["ln1_g"], np.float32)
                       ).reshape(2, 128).T
    biasp[:, 12:14] = (16.0 * np.asarray(inputs["ln1_b"], np.float32)
                       ).reshape(2, 128).T
    biasp[:, 14:16] = np.asarray(inputs["ln2_g"], np.float32).reshape(2, 128).T
    biasp[:, 16:18] = np.asarray(inputs["ln2_b"], np.float32).reshape(2, 128).T

    shared = dict(
        A8=_f8(A8), wv8=_f8(wv8), wo8=_f8(wo8), ff18=_f8(ff18),
        ff28=_f8(ff28), biasp=biasp,
    )
    wo_b = np.asarray(inputs["wo_b"], np.float32)
    in_maps = []
    for b in range(B):
        xT = np.ascontiguousarray(x[b].T)  # [D, S]
        m = dict(shared)
        m["x8"] = _f8((16.0 * xT).reshape(2, 128, S).transpose(1, 0, 2))
        m["xres"] = (xT + wo_b[:, None]).reshape(ND, 128, S).astype(
            ml_dtypes.bfloat16)
        in_maps.append(m)
    return in_maps


def run_sharded(inputs, trace=False, trace_kwargs=None):
    if "nc" not in _CACHE:
        _CACHE["nc"] = _build()
    nc = _CACHE["nc"]
    in_maps = _prep_inputs(inputs)
    res = run_bass_kernel_spmd(nc, in_maps, list(range(N_CORES)), trace=trace,
                               **(trace_kwargs or {}))
    outs = []
    for b in range(B):
        r = np.asarray(res.results[b]["out"], np.float32).reshape(D, S)
        outs.append(r.T)
    return np.stack(outs), res


def kernel(**inputs) -> np.ndarray:
    mask = np.asarray(inputs["attention_mask"])
    if not np.all(mask != 0):
        return _np_reference(**{k: np.asarray(v) for k, v in inputs.items()})
    out, _ = run_sharded(inputs, trace=False)
    return out
